# revision 10
# baseline (speedup 1.0000x reference)
"""Trainium2 Bass kernel for nn_Attention_13864154431876.

Dense transformer attention block: QKV projection + RoPE + causal GQA
attention (32 q heads, 8 kv heads, head_dim 128) + output projection.
B=2, S=2048, D=4096, start_pos=0 (cache fully overwritten).

Sharding (8 NeuronCores, tensor parallel by attention heads):
  - each core owns 4 q-heads and 1 kv-head (wq/wk/wv output-dim shards)
  - x is replicated (shipped pre-transposed as x^T so the contraction dim
    lands on partitions)
  - after attention, an on-chip AllToAll redistributes attn^T from
    head-sharded to token-sharded; each core then multiplies its 512-token
    slab against the full wo and the host concatenates the 8 slabs.

All matmuls run in float32r (hardware fast-fp32: operands rounded to
11-bit mantissa, exact fp32 accumulate) at 1 cycle/row.
"""
import sys

sys.path.insert(0, "/root/.axon_site/_ro/trn_rl_repo")

import numpy as np

import concourse.bass as bass
import concourse.mybir as mybir
import concourse.tile as tile
from concourse import bacc
from concourse.bass_utils import run_bass_kernel_spmd

F32 = mybir.dt.float32
F32R = mybir.dt.float32r
AF = mybir.ActivationFunctionType
ALU = mybir.AluOpType

N_CORES = 8
B, S, D = 2, 2048, 4096
H, KH, HD = 32, 8, 128
MS = 2048                     # max_seq_len (cache length)
BS = B * S                    # flattened tokens, b-major
HPC = H // N_CORES            # q-heads per core = 4
QF = HPC * HD                 # per-core q-feature width = 512
TB = 512                      # token block
NTB = BS // TB                # 8 token blocks
QBPB = S // TB                # 4 q-blocks per batch element
KC = D // 128                 # 32 contraction chunks
JCB = S // 128                # 16 j-chunks per batch element
SCALE = 1.0 / np.sqrt(HD)
TOKS_PER_CORE = BS // N_CORES  # 512


def round_fp32r(x: np.ndarray) -> np.ndarray:
    """Round fp32 -> fp32r bits (11-bit mantissa, round-to-nearest-even)."""
    u = np.ascontiguousarray(x, dtype=np.float32).view(np.uint32)
    lsb = (u >> 12) & 1
    return ((u + np.uint32(0x7FF) + lsb) & np.uint32(0xFFFFF000)).view(np.float32)


def build_attn_nc():
    nc = bacc.Bacc("TRN2", target_bir_lowering=False, debug=False,
                   num_devices=N_CORES)

    # ---- DRAM I/O ----------------------------------------------------
    xt_d = nc.dram_tensor("xt", [D, BS], F32R, kind="ExternalInput").ap()
    wq_d = nc.dram_tensor("wq", [D, QF], F32R, kind="ExternalInput").ap()
    wk_d = nc.dram_tensor("wk", [D, HD], F32R, kind="ExternalInput").ap()
    wv_d = nc.dram_tensor("wv", [D, HD], F32R, kind="ExternalInput").ap()
    wo_d = nc.dram_tensor("wo", [D, D], F32R, kind="ExternalInput").ap()
    cos_d = nc.dram_tensor("cosT", [HD, S], F32, kind="ExternalInput").ap()
    sin_d = nc.dram_tensor("sinT", [HD, S], F32, kind="ExternalInput").ap()
    mask_d = nc.dram_tensor("maskd", [128, 4, TB], F32, kind="ExternalInput").ap()
    rot_d = nc.dram_tensor("rotm", [HD, HD], F32R, kind="ExternalInput").ap()
    ident_d = nc.dram_tensor("ident", [128, 128], F32, kind="ExternalInput").ap()
    onesc_d = nc.dram_tensor("onesc", [128, 1], F32R, kind="ExternalInput").ap()
    onesr_d = nc.dram_tensor("onesr", [1, 128], F32R, kind="ExternalInput").ap()
    y_d = nc.dram_tensor("y", [TOKS_PER_CORE, D], F32, kind="ExternalOutput").ap()

    # internal DRAM for the collective
    attn_loc = nc.dram_tensor("attn_loc", [BS, TB], F32R)  # [8 chunks x 512hf, 512tok]
    attn_g = nc.dram_tensor("attn_g", [D, TB], F32R)

    with tile.TileContext(nc) as tc:
        # ---- persistent tiles ---------------------------------------
        persist_cm = tc.tile_pool(name="persist", bufs=1)
        persist = persist_cm.__enter__()
        wq_sb = persist.tile([128, KC, QF], F32R, name="wq_sb")
        wk_sb = persist.tile([128, KC, HD], F32R, name="wk_sb")
        wv_sb = persist.tile([128, KC, HD], F32R, name="wv_sb")
        cos_sb = persist.tile([HD, S], F32, name="cos_sb")
        sin_sb = persist.tile([HD, S], F32, name="sin_sb")
        mask_sb = persist.tile([128, 4, TB], F32, name="mask_sb")
        rot_sb = persist.tile([HD, HD], F32R, name="rot_sb")
        ident_sb = persist.tile([128, 128], F32, name="ident_sb")
        onesc_sb = persist.tile([128, 1], F32R, name="onesc_sb")
        onesr_sb = persist.tile([1, 128], F32R, name="onesr_sb")
        kt_sb = [persist.tile([HD, S], F32R, name=f"kt{b}_sb") for b in range(B)]
        v_sb = [persist.tile([128, JCB, HD], F32R, name=f"v{b}_sb") for b in range(B)]

        nc.sync.dma_start(wq_sb[:], wq_d.rearrange("(kc p) n -> p kc n", p=128))
        nc.sync.dma_start(wk_sb[:], wk_d.rearrange("(kc p) n -> p kc n", p=128))
        nc.sync.dma_start(wv_sb[:], wv_d.rearrange("(kc p) n -> p kc n", p=128))
        nc.sync.dma_start(cos_sb[:], cos_d[:])
        nc.sync.dma_start(sin_sb[:], sin_d[:])
        nc.sync.dma_start(mask_sb[:], mask_d[:])
        nc.sync.dma_start(rot_sb[:], rot_d[:])
        nc.sync.dma_start(ident_sb[:], ident_d[:])
        nc.sync.dma_start(onesc_sb[:], onesc_d[:])
        nc.sync.dma_start(onesr_sb[:], onesr_d[:])

        # ---- phase 1+2 pools ----------------------------------------
        p12 = []

        def pool12(*a, **kw):
            cm = tc.tile_pool(*a, **kw)
            p = cm.__enter__()
            p12.append((cm, p))
            return p

        xtp = pool12(name="xtp", bufs=3)
        qrawp = pool12(name="qrawp", bufs=2)
        vtrawp = pool12(name="vtrawp", bufs=2)
        qtp = pool12(name="qtp", bufs=6)
        tmpp = pool12(name="tmpp", bufs=3)
        ptp = pool12(name="ptp", bufs=3)
        denp = pool12(name="denp", bufs=3)
        recp = pool12(name="recp", bufs=2)
        recbp = pool12(name="recbp", bufs=1)
        atp = pool12(name="atp", bufs=2)
        pp = pool12(name="pp", bufs=6, space="PSUM")
        ps = pool12(name="ps", bufs=1, space="PSUM")
        pa = pool12(name="pa", bufs=1, space="PSUM")

        def emit_attention(tb, ps_pool, pa_pool, qt_tiles):
            """Generator: attention for token block tb. Yields between steps."""
            b, qb = tb // QBPB, tb % QBPB
            njc = (qb + 1) * 4
            s0 = qb * TB
            for h in range(HPC):
                denacc = denp.tile([128, TB], F32, name="denacc", tag="den")
                denf = denp.tile([128, TB], F32R, name="denf", tag="den")
                aps = pa_pool.tile([128, TB], F32, name="aps", tag="aps")
                pts = []
                for jc in range(njc):
                    sps = ps_pool.tile([128, TB], F32, name="sps", tag="sps")
                    nc.tensor.matmul(
                        sps[:], kt_sb[b][:, jc * 128:(jc + 1) * 128],
                        qt_tiles[h][:], start=True, stop=True,
                        skip_group_check=True)
                    r = jc - qb * 4
                    pt = ptp.tile([128, TB], F32R, name="pt", tag="pt")
                    if r >= 0:
                        praw = tmpp.tile([128, TB], F32, name="praw", tag="tmp")
                        nc.scalar.activation(praw[:], sps[:], AF.Exp)
                        nc.vector.tensor_tensor(pt[:], praw[:],
                                                mask_sb[:, r, :], ALU.mult)
                    else:
                        nc.scalar.activation(pt[:], sps[:], AF.Exp)
                    # denominator accumulation (fp32; final add rounds to f32r)
                    if jc == 0:
                        nc.vector.tensor_copy(denacc[:], pt[:].bitcast(F32))
                    elif jc == njc - 1:
                        nc.vector.tensor_tensor(denf[:], denacc[:],
                                                pt[:].bitcast(F32), ALU.add)
                    else:
                        nc.vector.tensor_tensor(denacc[:], denacc[:],
                                                pt[:].bitcast(F32), ALU.add)
                    pts.append(pt)
                    # AV accumulation (one step behind)
                    nc.tensor.matmul(
                        aps[:], v_sb[b][:, jc, :], pt[:],
                        start=(jc == 0), stop=(jc == njc - 1),
                        skip_group_check=True)
                    yield
                # denominator -> reciprocal -> broadcast -> scale
                csum = ps_pool.tile([1, TB], F32, name="csum", tag="sps")
                nc.tensor.matmul(csum[:], onesc_sb[:], denf[:],
                                 start=True, stop=True, skip_group_check=True)
                recip = recp.tile([1, TB], F32R, name="recip", tag="rec")
                with nc.allow_low_precision(reason="f32r softmax denom"):
                    nc.vector.reciprocal(recip[:], csum[:])
                bcast = ps_pool.tile([128, TB], F32, name="bcast", tag="sps")
                nc.tensor.matmul(bcast[:], onesr_sb[:], recip[:],
                                 start=True, stop=True, skip_group_check=True)
                recipb = recbp.tile([128, TB], F32, name="recipb", tag="recb")
                nc.vector.tensor_copy(recipb[:], bcast[:])
                attn_t = atp.tile([128, TB], F32R, name="attn_t", tag="attn_t")
                nc.vector.tensor_tensor(attn_t[:], aps[:], recipb[:], ALU.mult)
                nc.sync.dma_start(
                    attn_loc[tb * TB + h * 128: tb * TB + (h + 1) * 128, :],
                    attn_t[:])
                yield

        def drive(gen, n):
            if gen is None:
                return None
            for _ in range(n):
                try:
                    next(gen)
                except StopIteration:
                    return None
            return gen

        prev_gen = None
        prev_steps = 0
        for tb in range(NTB):
            b, qb = tb // QBPB, tb % QBPB
            s0 = qb * TB
            per_kc = max(1, -(-prev_steps // KC))  # ceil
            # ---- projections for tb, interleaved with attention(tb-1)
            qps = [pp.tile([128, TB], F32, name=f"qps{h}", tag="proj")
                   for h in range(HPC)]
            kps = pp.tile([128, TB], F32, name="kps", tag="proj")
            vtps = pp.tile([128, TB], F32, name="vtps", tag="proj")
            for kc in range(KC):
                xt_t = xtp.tile([128, TB], F32R, name="xt_t", tag="xt")
                nc.sync.dma_start(
                    xt_t[:], xt_d[kc * 128:(kc + 1) * 128,
                                  tb * TB:(tb + 1) * TB])
                for h in range(HPC):
                    nc.tensor.matmul(
                        qps[h][:], wq_sb[:, kc, h * 128:(h + 1) * 128],
                        xt_t[:], start=(kc == 0), stop=(kc == KC - 1),
                        skip_group_check=True)
                nc.tensor.matmul(kps[:], wk_sb[:, kc, :], xt_t[:],
                                 start=(kc == 0), stop=(kc == KC - 1),
                                 skip_group_check=True)
                nc.tensor.matmul(vtps[:], wv_sb[:, kc, :], xt_t[:],
                                 start=(kc == 0), stop=(kc == KC - 1),
                                 skip_group_check=True)
                prev_gen = drive(prev_gen, per_kc)

            # ---- drains + RoPE + V transpose ------------------------
            qt_tiles = []
            for h in range(HPC):
                qraw = qrawp.tile([128, TB], F32R, name="qraw", tag="qraw")
                nc.vector.tensor_copy(qraw[:], qps[h][:])
                rotps = ps.tile([128, TB], F32, name="rotps", tag="sps")
                nc.tensor.matmul(rotps[:], rot_sb[:], qraw[:],
                                 start=True, stop=True, skip_group_check=True)
                tcos = tmpp.tile([128, TB], F32, name="tcos", tag="tmp")
                nc.vector.tensor_tensor(tcos[:], qraw[:].bitcast(F32),
                                        cos_sb[:, s0:s0 + TB], ALU.mult)
                tsin = tmpp.tile([128, TB], F32, name="tsin", tag="tmp")
                nc.vector.tensor_tensor(tsin[:], rotps[:],
                                        sin_sb[:, s0:s0 + TB], ALU.mult)
                qt = qtp.tile([128, TB], F32R, name="qt", tag="qt")
                nc.vector.tensor_tensor(qt[:], tcos[:], tsin[:], ALU.add)
                qt_tiles.append(qt)
                prev_gen = drive(prev_gen, 1)
            # K
            kraw = qrawp.tile([128, TB], F32R, name="kraw", tag="qraw")
            nc.vector.tensor_copy(kraw[:], kps[:])
            rotps = ps.tile([128, TB], F32, name="rotpsk", tag="sps")
            nc.tensor.matmul(rotps[:], rot_sb[:], kraw[:],
                             start=True, stop=True, skip_group_check=True)
            tcos = tmpp.tile([128, TB], F32, name="tcosk", tag="tmp")
            nc.vector.tensor_tensor(tcos[:], kraw[:].bitcast(F32),
                                    cos_sb[:, s0:s0 + TB], ALU.mult)
            tsin = tmpp.tile([128, TB], F32, name="tsink", tag="tmp")
            nc.vector.tensor_tensor(tsin[:], rotps[:],
                                    sin_sb[:, s0:s0 + TB], ALU.mult)
            nc.vector.tensor_tensor(kt_sb[b][:, s0:s0 + TB], tcos[:],
                                    tsin[:], ALU.add)
            # V: drain V^T then transpose 4x [128,128]
            vtraw = vtrawp.tile([128, TB], F32, name="vtraw", tag="vtraw")
            nc.vector.tensor_copy(vtraw[:], vtps[:])
            vtr = pp.tile([128, TB], F32, name="vtr", tag="proj")
            for t4 in range(4):
                nc.tensor.transpose(vtr[:, t4 * 128:(t4 + 1) * 128],
                                    vtraw[:, t4 * 128:(t4 + 1) * 128],
                                    ident_sb[:])
            nc.vector.tensor_copy(
                v_sb[b].rearrange("p jc d -> p (jc d)")[:, s0:s0 + TB],
                vtr[:])
            prev_gen = drive(prev_gen, 10 ** 9)  # flush any leftovers
            if tb < NTB - 1:
                prev_gen = emit_attention(tb, ps, pa, qt_tiles)
                prev_steps = HPC * ((qb + 1) * 4 + 1)
            else:
                tail_qt = qt_tiles

        # close phase-1/2 psum pools, then run the attention tail (tb=7)
        # with more generous buffering
        psum_entries = [e for e in p12 if e[1] in (pp, ps, pa)]
        for cm, p in reversed(psum_entries):
            p12.remove((cm, p))
            cm.__exit__(None, None, None)
        ps2_cm = tc.tile_pool(name="ps2", bufs=4, space="PSUM")
        ps2 = ps2_cm.__enter__()
        pa2_cm = tc.tile_pool(name="pa2", bufs=2, space="PSUM")
        pa2 = pa2_cm.__enter__()
        drive(emit_attention(NTB - 1, ps2, pa2, tail_qt), 10 ** 9)
        pa2_cm.__exit__(None, None, None)
        ps2_cm.__exit__(None, None, None)

        for cm, p in reversed(p12):
            cm.__exit__(None, None, None)
        persist_cm.__exit__(None, None, None)

        # ---- AllToAll: head-sharded -> token-sharded ----------------
        nc.gpsimd.collective_compute(
            "AllToAll", ALU.bypass,
            replica_groups=[list(range(N_CORES))],
            ins=[attn_loc.ap().opt()], outs=[attn_g.ap().opt()],
        )

        # ---- phase 3: y = attn_rows @ wo ----------------------------
        with tc.tile_pool(name="attn_sb", bufs=1) as ap3, \
             tc.tile_pool(name="wop", bufs=10) as wop, \
             tc.tile_pool(name="ysb", bufs=4) as ysbp, \
             tc.tile_pool(name="py", bufs=4, space="PSUM") as pyp:
            attn_sb = []
            for tc4 in range(4):
                t = ap3.tile([128, KC, 128], F32R, name=f"attn_sb{tc4}")
                nc.sync.dma_start(
                    t[:], attn_g.rearrange("(hc p) q -> p hc q", p=128)
                    [:, :, tc4 * 128:(tc4 + 1) * 128])
                attn_sb.append(t)
            for ob in range(8):
                wo_g = []
                for g in range(8):
                    wt = wop.tile([128, 4, TB], F32R, name="wo_t", tag="wo")
                    nc.sync.dma_start(
                        wt[:], wo_d.rearrange("(hc p) n -> p hc n", p=128)
                        [:, g * 4:(g + 1) * 4, ob * TB:(ob + 1) * TB])
                    wo_g.append(wt)
                for tc4 in range(4):
                    yps = pyp.tile([128, TB], F32, name="yps", tag="yps")
                    for hc in range(KC):
                        nc.tensor.matmul(
                            yps[:], attn_sb[tc4][:, hc, :],
                            wo_g[hc // 4][:, hc % 4, :],
                            start=(hc == 0), stop=(hc == KC - 1),
                            skip_group_check=True)
                    y_sb = ysbp.tile([128, TB], F32, name="y_sb", tag="y")
                    nc.vector.tensor_copy(y_sb[:], yps[:])
                    nc.sync.dma_start(
                        y_d[tc4 * 128:(tc4 + 1) * 128,
                            ob * TB:(ob + 1) * TB], y_sb[:])

    nc.compile()
    return nc


_NC_CACHE = None


def _get_nc():
    global _NC_CACHE
    if _NC_CACHE is None:
        _NC_CACHE = build_attn_nc()
    return _NC_CACHE


def _host_reference(x, wq, wk, wv, wo, sincos, start_pos, causal_mask):
    """Numpy fallback (only used if the mask is not causal-tril)."""
    xq = (x @ wq).reshape(B, S, H, HD)
    xk = (x @ wk).reshape(B, S, KH, HD)
    xv = (x @ wv).reshape(B, S, KH, HD)
    sp = min(max(int(start_pos), 0), MS - S)
    sc = sincos[sp:sp + S]
    sin, cos = sc[:, :HD], sc[:, HD:]
    sin = sin[None, :, None, :]
    cos = cos[None, :, None, :]

    def rot(u):
        return np.concatenate([-u[..., HD // 2:], u[..., :HD // 2]], axis=-1)

    xq = xq * cos + rot(xq) * sin
    xk = xk * cos + rot(xk) * sin
    mask = np.broadcast_to(causal_mask[:, sp:sp + S, :MS], (B, S, MS))
    out = np.zeros((B, S, H, HD), dtype=np.float32)
    nrep = H // KH
    for b in range(B):
        for h in range(H):
            q = xq[b, :, h]
            k = xk[b, :, h // nrep]
            v = xv[b, :, h // nrep]
            s = (q @ k.T) * SCALE
            s = np.where(mask[b], s, -np.inf)
            s = s - s.max(axis=-1, keepdims=True)
            p = np.exp(s)
            p /= p.sum(axis=-1, keepdims=True)
            out[b, :, h] = p @ v
    return out.reshape(B, S, H * HD) @ wo


def kernel(x, wq, wk, wv, wo, cache_k, cache_v, sincos, causal_mask,
           start_pos):
    x = np.asarray(x, dtype=np.float32)
    wq = np.asarray(wq, dtype=np.float32)
    wk = np.asarray(wk, dtype=np.float32)
    wv = np.asarray(wv, dtype=np.float32)
    wo = np.asarray(wo, dtype=np.float32)
    sincos = np.asarray(sincos, dtype=np.float32)
    cm = np.asarray(causal_mask)
    sp = min(max(int(start_pos), 0), MS - S)

    tril = np.tril(np.ones((S, MS), dtype=bool))
    if not np.array_equal(cm[0, sp:sp + S, :], tril[:, :MS]):
        return _host_reference(x, wq, wk, wv, wo, sincos, start_pos,
                               cm).astype(np.float32)

    # host prep
    sc = sincos[sp:sp + S]
    sinT = np.ascontiguousarray(sc[:, :HD].T)       # [HD, S]
    cosT = np.ascontiguousarray(sc[:, HD:].T)       # [HD, S]
    xt = round_fp32r(np.ascontiguousarray(x.reshape(BS, D).T))
    wqs = wq * np.float32(SCALE)
    wo_r = round_fp32r(wo)

    maskd = np.zeros((128, 4, TB), dtype=np.float32)
    j = np.arange(128)[:, None, None]
    r = np.arange(4)[None, :, None]
    q = np.arange(TB)[None, None, :]
    maskd[(r * 128 + j) <= q] = 1.0

    rotm = np.zeros((HD, HD), dtype=np.float32)
    hh = HD // 2
    rotm[np.arange(hh) + hh, np.arange(hh)] = -1.0
    rotm[np.arange(hh), np.arange(hh) + hh] = 1.0

    ident = np.eye(128, dtype=np.float32)
    onesc = np.ones((128, 1), dtype=np.float32)
    onesr = np.ones((1, 128), dtype=np.float32)

    in_maps = []
    for c in range(N_CORES):
        in_maps.append({
            "xt": xt,
            "wq": round_fp32r(wqs[:, c * QF:(c + 1) * QF]),
            "wk": round_fp32r(wk[:, c * HD:(c + 1) * HD]),
            "wv": round_fp32r(wv[:, c * HD:(c + 1) * HD]),
            "wo": wo_r,
            "cosT": cosT, "sinT": sinT,
            "maskd": maskd, "rotm": rotm, "ident": ident,
            "onesc": onesc, "onesr": onesr,
        })

    global _LAST_IN_MAPS
    _LAST_IN_MAPS = in_maps
    nc = _get_nc()
    res = run_bass_kernel_spmd(nc, in_maps, list(range(N_CORES)))
    y = np.concatenate([res.results[c]["y"] for c in range(N_CORES)], axis=0)
    return y.reshape(B, S, D).astype(np.float32)


# revision 17
# speedup vs baseline: 1.1205x; 1.1205x over previous
"""Trainium2 Bass kernel for nn_Attention_13864154431876.

Dense transformer attention block: QKV projection + RoPE + causal GQA
attention (32 q heads, 8 kv heads, head_dim 128) + output projection.
B=2, S=2048, D=4096, start_pos=0 (cache fully overwritten).

Sharding (8 NeuronCores, tensor parallel by attention heads):
  - each core owns 4 q-heads and 1 kv-head (wq/wk/wv output-dim shards)
  - x is replicated (shipped pre-transposed as x^T so the contraction dim
    lands on partitions)
  - after attention, an on-chip AllToAll redistributes attn^T from
    head-sharded to token-sharded; each core then multiplies its 512-token
    slab against the full wo and the host concatenates the 8 slabs.

All matmuls run in float32r (hardware fast-fp32: operands rounded to
11-bit mantissa, exact fp32 accumulate) at 1 cycle/row.
"""
import sys

sys.path.insert(0, "/root/.axon_site/_ro/trn_rl_repo")

import numpy as np

import concourse.bass as bass
import concourse.mybir as mybir
import concourse.tile as tile
from concourse import bacc
from concourse.bass_utils import run_bass_kernel_spmd

F32 = mybir.dt.float32
F32R = mybir.dt.float32r
AF = mybir.ActivationFunctionType
ALU = mybir.AluOpType

N_CORES = 8
B, S, D = 2, 2048, 4096
H, KH, HD = 32, 8, 128
MS = 2048                     # max_seq_len (cache length)
BS = B * S                    # flattened tokens, b-major
HPC = H // N_CORES            # q-heads per core = 4
QF = HPC * HD                 # per-core q-feature width = 512
TB = 512                      # token block
NTB = BS // TB                # 8 token blocks
QBPB = S // TB                # 4 q-blocks per batch element
KC = D // 128                 # 32 contraction chunks
JCB = S // 128                # 16 j-chunks per batch element
SCALE = 1.0 / np.sqrt(HD)
TOKS_PER_CORE = BS // N_CORES  # 512


def round_fp32r(x: np.ndarray) -> np.ndarray:
    """Round fp32 -> fp32r bits (11-bit mantissa, round-to-nearest-even)."""
    u = np.ascontiguousarray(x, dtype=np.float32).view(np.uint32)
    lsb = (u >> 12) & 1
    return ((u + np.uint32(0x7FF) + lsb) & np.uint32(0xFFFFF000)).view(np.float32)


def build_attn_nc(mock_collectives=False):
    nc = bacc.Bacc("TRN2", target_bir_lowering=False, debug=False,
                   num_devices=N_CORES)

    # ---- DRAM I/O ----------------------------------------------------
    xt_d = nc.dram_tensor("xt", [D, BS], F32R, kind="ExternalInput").ap()
    wq_d = nc.dram_tensor("wq", [D, QF], F32R, kind="ExternalInput").ap()
    wk_d = nc.dram_tensor("wk", [D, HD], F32R, kind="ExternalInput").ap()
    wv_d = nc.dram_tensor("wv", [D, HD], F32R, kind="ExternalInput").ap()
    wo_d = nc.dram_tensor("wo", [D, D], F32R, kind="ExternalInput").ap()
    cos_d = nc.dram_tensor("cosT", [HD, S], F32, kind="ExternalInput").ap()
    sin_d = nc.dram_tensor("sinT", [HD, S], F32, kind="ExternalInput").ap()
    mask_d = nc.dram_tensor("maskd", [128, 4, TB], F32, kind="ExternalInput").ap()
    rot_d = nc.dram_tensor("rotm", [HD, HD], F32R, kind="ExternalInput").ap()
    ident_d = nc.dram_tensor("ident", [128, 128], F32, kind="ExternalInput").ap()
    ones_d = nc.dram_tensor("ones128", [128, 128], F32R, kind="ExternalInput").ap()
    y_d = nc.dram_tensor("y", [TOKS_PER_CORE, D], F32, kind="ExternalOutput").ap()

    # internal DRAM for the two per-batch-element AllToAlls.
    # attn_locX rows are chunk-major: chunk j (512 rows) = my 512 head-feats
    # for 256-token group j of batch element X. After A2A, attn_gX rows are
    # global head-feats for MY 256-token slab of batch element X.
    HTB = TB // 2  # 256
    attn_loc = [nc.dram_tensor(f"attn_loc{b}", [BS, HTB], F32R) for b in range(B)]
    attn_g = [nc.dram_tensor(f"attn_g{b}", [D, HTB], F32R) for b in range(B)]

    with tile.TileContext(nc) as tc:
        # ---- persistent tiles ---------------------------------------
        persist_cm = tc.tile_pool(name="persist", bufs=1)
        persist = persist_cm.__enter__()
        wq_sb = persist.tile([128, KC, QF], F32R, name="wq_sb")
        wk_sb = persist.tile([128, KC, HD], F32R, name="wk_sb")
        wv_sb = persist.tile([128, KC, HD], F32R, name="wv_sb")
        cos_sb = persist.tile([HD, S], F32, name="cos_sb")
        sin_sb = persist.tile([HD, S], F32, name="sin_sb")
        mask_sb = persist.tile([128, 4, TB], F32, name="mask_sb")
        rot_sb = persist.tile([HD, HD], F32R, name="rot_sb")
        ident_sb = persist.tile([128, 128], F32, name="ident_sb")
        ones_sb = persist.tile([128, 128], F32R, name="ones_sb")
        kt_sb = [persist.tile([HD, S], F32R, name=f"kt{b}_sb") for b in range(B)]
        v_sb = [persist.tile([128, JCB, HD], F32R, name=f"v{b}_sb") for b in range(B)]

        nc.sync.dma_start(wq_sb[:], wq_d.rearrange("(kc p) n -> p kc n", p=128))
        nc.sync.dma_start(wk_sb[:], wk_d.rearrange("(kc p) n -> p kc n", p=128))
        nc.sync.dma_start(wv_sb[:], wv_d.rearrange("(kc p) n -> p kc n", p=128))
        nc.sync.dma_start(cos_sb[:], cos_d[:])
        nc.sync.dma_start(sin_sb[:], sin_d[:])
        nc.sync.dma_start(mask_sb[:], mask_d[:])
        nc.sync.dma_start(rot_sb[:], rot_d[:])
        nc.sync.dma_start(ident_sb[:], ident_d[:])
        nc.sync.dma_start(ones_sb[:], ones_d[:])

        # ---- phase 1+2 pools ----------------------------------------
        p12 = []

        def pool12(*a, **kw):
            cm = tc.tile_pool(*a, **kw)
            p = cm.__enter__()
            p12.append((cm, p))
            return p

        xtp = pool12(name="xtp", bufs=3)
        qrawp = pool12(name="qrawp", bufs=2)
        vtrawp = pool12(name="vtrawp", bufs=2)
        qtp = pool12(name="qtp", bufs=6)
        tmpp = pool12(name="tmpp", bufs=3)
        ptp = pool12(name="ptp", bufs=3)
        denp = pool12(name="denp", bufs=3)
        recbp = pool12(name="recbp", bufs=1)
        atp = pool12(name="atp", bufs=2)
        asbp = pool12(name="asbp", bufs=2)
        pp = pool12(name="pp", bufs=6, space="PSUM")
        ps = pool12(name="ps", bufs=1, space="PSUM")
        pa = pool12(name="pa", bufs=1, space="PSUM")

        def emit_attention(tb, ps_pool, pa_pool, qt_tiles):
            """Generator: attention for token block tb. Yields between steps."""
            b, qb = tb // QBPB, tb % QBPB
            njc = (qb + 1) * 4
            s0 = qb * TB
            for h in range(HPC):
                denacc = denp.tile([128, TB], F32, name="denacc", tag="den")
                denf = denp.tile([128, TB], F32R, name="denf", tag="den")
                aps = pa_pool.tile([128, TB], F32, name="aps", tag="aps")
                pts = []
                for jc in range(njc):
                    sps = ps_pool.tile([128, TB], F32, name="sps", tag="sps")
                    nc.tensor.matmul(
                        sps[:], kt_sb[b][:, jc * 128:(jc + 1) * 128],
                        qt_tiles[h][:], start=True, stop=True,
                        skip_group_check=True)
                    r = jc - qb * 4
                    pt = ptp.tile([128, TB], F32R, name="pt", tag="pt")
                    if r >= 0:
                        praw = tmpp.tile([128, TB], F32, name="praw", tag="tmp")
                        nc.scalar.activation(praw[:], sps[:], AF.Exp)
                        nc.vector.tensor_tensor(pt[:], praw[:],
                                                mask_sb[:, r, :], ALU.mult)
                    else:
                        nc.scalar.activation(pt[:], sps[:], AF.Exp)
                    # denominator accumulation (fp32; final add rounds to f32r)
                    if jc == 0:
                        nc.vector.tensor_copy(denacc[:], pt[:].bitcast(F32))
                    elif jc == njc - 1:
                        nc.vector.tensor_tensor(denf[:], denacc[:],
                                                pt[:].bitcast(F32), ALU.add)
                    else:
                        nc.vector.tensor_tensor(denacc[:], denacc[:],
                                                pt[:].bitcast(F32), ALU.add)
                    pts.append(pt)
                    # AV accumulation (one step behind)
                    nc.tensor.matmul(
                        aps[:], v_sb[b][:, jc, :], pt[:],
                        start=(jc == 0), stop=(jc == njc - 1),
                        skip_group_check=True)
                    yield
                # free the attn-accumulator bank early, then
                # colsum+broadcast in one ones-matmul and a fast reciprocal
                asb = asbp.tile([128, TB], F32, name="asb", tag="asb")
                nc.vector.tensor_copy(asb[:], aps[:])
                denb = ps_pool.tile([128, TB], F32, name="denb", tag="sps")
                nc.tensor.matmul(denb[:], ones_sb[:], denf[:],
                                 start=True, stop=True, skip_group_check=True)
                recipb = recbp.tile([128, TB], F32, name="recipb", tag="recb")
                nc.vector.reciprocal_approx_fast(recipb[:], denb[:])
                attn_t = atp.tile([128, TB], F32R, name="attn_t", tag="attn_t")
                nc.vector.tensor_tensor(attn_t[:], asb[:], recipb[:], ALU.mult)
                lb = tb % QBPB  # local 512-token block within batch element b
                for half in range(2):
                    nc.sync.dma_start(
                        attn_loc[b].ap()[
                            (2 * lb + half) * 512 + h * 128:
                            (2 * lb + half) * 512 + (h + 1) * 128, :],
                        attn_t[:, half * HTB:(half + 1) * HTB])
                yield

        def drive(gen, n):
            if gen is None:
                return None
            for _ in range(n):
                try:
                    next(gen)
                except StopIteration:
                    return None
            return gen

        prev_gen = None
        prev_steps = 0
        for tb in range(NTB):
            b, qb = tb // QBPB, tb % QBPB
            s0 = qb * TB
            per_kc = max(1, -(-prev_steps // KC))  # ceil
            # ---- projections for tb, interleaved with attention(tb-1)
            qps = [pp.tile([128, TB], F32, name=f"qps{h}", tag="proj")
                   for h in range(HPC)]
            kps = pp.tile([128, TB], F32, name="kps", tag="proj")
            vtps = pp.tile([128, TB], F32, name="vtps", tag="proj")
            for kc in range(KC):
                xt_t = xtp.tile([128, TB], F32R, name="xt_t", tag="xt")
                nc.sync.dma_start(
                    xt_t[:], xt_d[kc * 128:(kc + 1) * 128,
                                  tb * TB:(tb + 1) * TB])
                for h in range(HPC):
                    nc.tensor.matmul(
                        qps[h][:], wq_sb[:, kc, h * 128:(h + 1) * 128],
                        xt_t[:], start=(kc == 0), stop=(kc == KC - 1),
                        skip_group_check=True)
                nc.tensor.matmul(kps[:], wk_sb[:, kc, :], xt_t[:],
                                 start=(kc == 0), stop=(kc == KC - 1),
                                 skip_group_check=True)
                nc.tensor.matmul(vtps[:], wv_sb[:, kc, :], xt_t[:],
                                 start=(kc == 0), stop=(kc == KC - 1),
                                 skip_group_check=True)
                prev_gen = drive(prev_gen, per_kc)

            # ---- drains + RoPE + V transpose ------------------------
            qt_tiles = []
            for h in range(HPC):
                qraw = qrawp.tile([128, TB], F32R, name="qraw", tag="qraw")
                nc.vector.tensor_copy(qraw[:], qps[h][:])
                rotps = ps.tile([128, TB], F32, name="rotps", tag="sps")
                nc.tensor.matmul(rotps[:], rot_sb[:], qraw[:],
                                 start=True, stop=True, skip_group_check=True)
                tcos = tmpp.tile([128, TB], F32, name="tcos", tag="tmp")
                nc.vector.tensor_tensor(tcos[:], qraw[:].bitcast(F32),
                                        cos_sb[:, s0:s0 + TB], ALU.mult)
                tsin = tmpp.tile([128, TB], F32, name="tsin", tag="tmp")
                nc.vector.tensor_tensor(tsin[:], rotps[:],
                                        sin_sb[:, s0:s0 + TB], ALU.mult)
                qt = qtp.tile([128, TB], F32R, name="qt", tag="qt")
                nc.vector.tensor_tensor(qt[:], tcos[:], tsin[:], ALU.add)
                qt_tiles.append(qt)
                prev_gen = drive(prev_gen, 1)
            # K
            kraw = qrawp.tile([128, TB], F32R, name="kraw", tag="qraw")
            nc.vector.tensor_copy(kraw[:], kps[:])
            rotps = ps.tile([128, TB], F32, name="rotpsk", tag="sps")
            nc.tensor.matmul(rotps[:], rot_sb[:], kraw[:],
                             start=True, stop=True, skip_group_check=True)
            tcos = tmpp.tile([128, TB], F32, name="tcosk", tag="tmp")
            nc.vector.tensor_tensor(tcos[:], kraw[:].bitcast(F32),
                                    cos_sb[:, s0:s0 + TB], ALU.mult)
            tsin = tmpp.tile([128, TB], F32, name="tsink", tag="tmp")
            nc.vector.tensor_tensor(tsin[:], rotps[:],
                                    sin_sb[:, s0:s0 + TB], ALU.mult)
            nc.vector.tensor_tensor(kt_sb[b][:, s0:s0 + TB], tcos[:],
                                    tsin[:], ALU.add)
            # V: drain V^T then transpose 4x [128,128]
            vtraw = vtrawp.tile([128, TB], F32, name="vtraw", tag="vtraw")
            nc.vector.tensor_copy(vtraw[:], vtps[:])
            vtr = pp.tile([128, TB], F32, name="vtr", tag="proj")
            for t4 in range(4):
                nc.tensor.transpose(vtr[:, t4 * 128:(t4 + 1) * 128],
                                    vtraw[:, t4 * 128:(t4 + 1) * 128],
                                    ident_sb[:])
            nc.vector.tensor_copy(
                v_sb[b].rearrange("p jc d -> p (jc d)")[:, s0:s0 + TB],
                vtr[:])
            prev_gen = drive(prev_gen, 10 ** 9)  # flush any leftovers
            if tb == QBPB:
                # all b=0 attention emitted -> overlap its AllToAll with
                # the remaining b=1 compute
                if mock_collectives:
                    nc.sync.dma_start(attn_g[0].ap()[:], attn_loc[0].ap()[:])
                else:
                    nc.gpsimd.collective_compute(
                        "AllToAll", ALU.bypass,
                        replica_groups=[list(range(N_CORES))],
                        ins=[attn_loc[0].ap().opt()],
                        outs=[attn_g[0].ap().opt()],
                    )
            if tb < NTB - 1:
                prev_gen = emit_attention(tb, ps, pa, qt_tiles)
                prev_steps = HPC * ((qb + 1) * 4 + 1)
            else:
                tail_qt = qt_tiles

        # close phase-1/2 psum pools, then run the attention tail (tb=7)
        # with more generous buffering
        psum_entries = [e for e in p12 if e[1] in (pp, ps, pa)]
        for cm, p in reversed(psum_entries):
            p12.remove((cm, p))
            cm.__exit__(None, None, None)
        ps2_cm = tc.tile_pool(name="ps2", bufs=4, space="PSUM")
        ps2 = ps2_cm.__enter__()
        pa2_cm = tc.tile_pool(name="pa2", bufs=2, space="PSUM")
        pa2 = pa2_cm.__enter__()
        drive(emit_attention(NTB - 1, ps2, pa2, tail_qt), 10 ** 9)
        pa2_cm.__exit__(None, None, None)
        ps2_cm.__exit__(None, None, None)

        for cm, p in reversed(p12):
            cm.__exit__(None, None, None)
        persist_cm.__exit__(None, None, None)

        # ---- AllToAll for b=1 ---------------------------------------
        if mock_collectives:
            nc.sync.dma_start(attn_g[1].ap()[:], attn_loc[1].ap()[:])
        else:
            nc.gpsimd.collective_compute(
                "AllToAll", ALU.bypass,
                replica_groups=[list(range(N_CORES))],
                ins=[attn_loc[1].ap().opt()], outs=[attn_g[1].ap().opt()],
            )

        # ---- phase 3: y = attn_rows @ wo ----------------------------
        with tc.tile_pool(name="attn_sb", bufs=1) as ap3, \
             tc.tile_pool(name="wop", bufs=10) as wop, \
             tc.tile_pool(name="ysb", bufs=4) as ysbp, \
             tc.tile_pool(name="py", bufs=4, space="PSUM") as pyp:
            attn_sb = []
            for tc4 in range(4):
                t = ap3.tile([128, KC, 128], F32R, name=f"attn_sb{tc4}")
                nc.sync.dma_start(
                    t[:], attn_g[tc4 // 2].ap()
                    .rearrange("(hc p) q -> p hc q", p=128)
                    [:, :, (tc4 % 2) * 128:(tc4 % 2 + 1) * 128])
                attn_sb.append(t)
            for ob in range(8):
                wo_g = []
                for g in range(8):
                    wt = wop.tile([128, 4, TB], F32R, name="wo_t", tag="wo")
                    nc.sync.dma_start(
                        wt[:], wo_d.rearrange("(hc p) n -> p hc n", p=128)
                        [:, g * 4:(g + 1) * 4, ob * TB:(ob + 1) * TB])
                    wo_g.append(wt)
                for tc4 in range(4):
                    yps = pyp.tile([128, TB], F32, name="yps", tag="yps")
                    for hc in range(KC):
                        nc.tensor.matmul(
                            yps[:], attn_sb[tc4][:, hc, :],
                            wo_g[hc // 4][:, hc % 4, :],
                            start=(hc == 0), stop=(hc == KC - 1),
                            skip_group_check=True)
                    y_sb = ysbp.tile([128, TB], F32, name="y_sb", tag="y")
                    nc.vector.tensor_copy(y_sb[:], yps[:])
                    nc.sync.dma_start(
                        y_d[tc4 * 128:(tc4 + 1) * 128,
                            ob * TB:(ob + 1) * TB], y_sb[:])

    nc.compile()
    return nc



_NC_CACHE = None


def _get_nc():
    global _NC_CACHE
    if _NC_CACHE is None:
        _NC_CACHE = build_attn_nc()
    return _NC_CACHE


def _host_reference(x, wq, wk, wv, wo, sincos, start_pos, causal_mask):
    """Numpy fallback (only used if the mask is not causal-tril)."""
    xq = (x @ wq).reshape(B, S, H, HD)
    xk = (x @ wk).reshape(B, S, KH, HD)
    xv = (x @ wv).reshape(B, S, KH, HD)
    sp = min(max(int(start_pos), 0), MS - S)
    sc = sincos[sp:sp + S]
    sin, cos = sc[:, :HD], sc[:, HD:]
    sin = sin[None, :, None, :]
    cos = cos[None, :, None, :]

    def rot(u):
        return np.concatenate([-u[..., HD // 2:], u[..., :HD // 2]], axis=-1)

    xq = xq * cos + rot(xq) * sin
    xk = xk * cos + rot(xk) * sin
    mask = np.broadcast_to(causal_mask[:, sp:sp + S, :MS], (B, S, MS))
    out = np.zeros((B, S, H, HD), dtype=np.float32)
    nrep = H // KH
    for b in range(B):
        for h in range(H):
            q = xq[b, :, h]
            k = xk[b, :, h // nrep]
            v = xv[b, :, h // nrep]
            s = (q @ k.T) * SCALE
            s = np.where(mask[b], s, -np.inf)
            s = s - s.max(axis=-1, keepdims=True)
            p = np.exp(s)
            p /= p.sum(axis=-1, keepdims=True)
            out[b, :, h] = p @ v
    return out.reshape(B, S, H * HD) @ wo


def kernel(x, wq, wk, wv, wo, cache_k, cache_v, sincos, causal_mask,
           start_pos):
    x = np.asarray(x, dtype=np.float32)
    wq = np.asarray(wq, dtype=np.float32)
    wk = np.asarray(wk, dtype=np.float32)
    wv = np.asarray(wv, dtype=np.float32)
    wo = np.asarray(wo, dtype=np.float32)
    sincos = np.asarray(sincos, dtype=np.float32)
    cm = np.asarray(causal_mask)
    sp = min(max(int(start_pos), 0), MS - S)

    tril = np.tril(np.ones((S, MS), dtype=bool))
    if not np.array_equal(cm[0, sp:sp + S, :], tril[:, :MS]):
        return _host_reference(x, wq, wk, wv, wo, sincos, start_pos,
                               cm).astype(np.float32)

    # host prep
    sc = sincos[sp:sp + S]
    sinT = np.ascontiguousarray(sc[:, :HD].T)       # [HD, S]
    cosT = np.ascontiguousarray(sc[:, HD:].T)       # [HD, S]
    xt = round_fp32r(np.ascontiguousarray(x.reshape(BS, D).T))
    wqs = wq * np.float32(SCALE)
    wo_r = round_fp32r(wo)

    maskd = np.zeros((128, 4, TB), dtype=np.float32)
    j = np.arange(128)[:, None, None]
    r = np.arange(4)[None, :, None]
    q = np.arange(TB)[None, None, :]
    maskd[(r * 128 + j) <= q] = 1.0

    rotm = np.zeros((HD, HD), dtype=np.float32)
    hh = HD // 2
    rotm[np.arange(hh) + hh, np.arange(hh)] = -1.0
    rotm[np.arange(hh), np.arange(hh) + hh] = 1.0

    ident = np.eye(128, dtype=np.float32)
    ones128 = np.ones((128, 128), dtype=np.float32)

    in_maps = []
    for c in range(N_CORES):
        in_maps.append({
            "xt": xt,
            "wq": round_fp32r(wqs[:, c * QF:(c + 1) * QF]),
            "wk": round_fp32r(wk[:, c * HD:(c + 1) * HD]),
            "wv": round_fp32r(wv[:, c * HD:(c + 1) * HD]),
            "wo": wo_r,
            "cosT": cosT, "sinT": sinT,
            "maskd": maskd, "rotm": rotm, "ident": ident,
            "ones128": ones128,
        })

    global _LAST_IN_MAPS
    _LAST_IN_MAPS = in_maps
    nc = _get_nc()
    res = run_bass_kernel_spmd(nc, in_maps, list(range(N_CORES)))
    # per-core y: rows [0:256] = b0 tokens c*256.., rows [256:512] = b1
    y = np.empty((BS, D), dtype=np.float32)
    half = TOKS_PER_CORE // 2
    for c in range(N_CORES):
        yc = res.results[c]["y"]
        y[c * half:(c + 1) * half] = yc[:half]
        y[S + c * half:S + (c + 1) * half] = yc[half:]
    return y.reshape(B, S, D)


# revision 18
# speedup vs baseline: 1.1258x; 1.0047x over previous
"""Trainium2 Bass kernel for nn_Attention_13864154431876.

Dense transformer attention block: QKV projection + RoPE + causal GQA
attention (32 q heads, 8 kv heads, head_dim 128) + output projection.
B=2, S=2048, D=4096, start_pos=0 (cache fully overwritten).

Sharding (8 NeuronCores, tensor parallel by attention heads):
  - each core owns 4 q-heads and 1 kv-head (wq/wk/wv output-dim shards)
  - x is replicated (shipped pre-transposed as x^T so the contraction dim
    lands on partitions)
  - after attention, an on-chip AllToAll redistributes attn^T from
    head-sharded to token-sharded; each core then multiplies its 512-token
    slab against the full wo and the host concatenates the 8 slabs.

All matmuls run in float32r (hardware fast-fp32: operands rounded to
11-bit mantissa, exact fp32 accumulate) at 1 cycle/row.
"""
import sys

sys.path.insert(0, "/root/.axon_site/_ro/trn_rl_repo")

import numpy as np

import concourse.bass as bass
import concourse.mybir as mybir
import concourse.tile as tile
from concourse import bacc
from concourse.bass_utils import run_bass_kernel_spmd

F32 = mybir.dt.float32
F32R = mybir.dt.float32r
AF = mybir.ActivationFunctionType
ALU = mybir.AluOpType

N_CORES = 8
B, S, D = 2, 2048, 4096
H, KH, HD = 32, 8, 128
MS = 2048                     # max_seq_len (cache length)
BS = B * S                    # flattened tokens, b-major
HPC = H // N_CORES            # q-heads per core = 4
QF = HPC * HD                 # per-core q-feature width = 512
TB = 512                      # token block
NTB = BS // TB                # 8 token blocks
QBPB = S // TB                # 4 q-blocks per batch element
KC = D // 128                 # 32 contraction chunks
JCB = S // 128                # 16 j-chunks per batch element
SCALE = 1.0 / np.sqrt(HD)
TOKS_PER_CORE = BS // N_CORES  # 512


def round_fp32r(x: np.ndarray) -> np.ndarray:
    """Round fp32 -> fp32r bits (11-bit mantissa, round-to-nearest-even)."""
    u = np.ascontiguousarray(x, dtype=np.float32).view(np.uint32)
    lsb = (u >> 12) & 1
    return ((u + np.uint32(0x7FF) + lsb) & np.uint32(0xFFFFF000)).view(np.float32)


def build_attn_nc(mock_collectives=False):
    nc = bacc.Bacc("TRN2", target_bir_lowering=False, debug=False,
                   num_devices=N_CORES)

    # ---- DRAM I/O ----------------------------------------------------
    xt_d = nc.dram_tensor("xt", [D, BS], F32R, kind="ExternalInput").ap()
    wq_d = nc.dram_tensor("wq", [D, QF], F32R, kind="ExternalInput").ap()
    wk_d = nc.dram_tensor("wk", [D, HD], F32R, kind="ExternalInput").ap()
    wv_d = nc.dram_tensor("wv", [D, HD], F32R, kind="ExternalInput").ap()
    wo_d = nc.dram_tensor("wo", [D, D], F32R, kind="ExternalInput").ap()
    cos_d = nc.dram_tensor("cosT", [HD, S], F32, kind="ExternalInput").ap()
    sin_d = nc.dram_tensor("sinT", [HD, S], F32, kind="ExternalInput").ap()
    mask_d = nc.dram_tensor("maskd", [128, 4, TB], F32, kind="ExternalInput").ap()
    rot_d = nc.dram_tensor("rotm", [HD, HD], F32R, kind="ExternalInput").ap()
    ident_d = nc.dram_tensor("ident", [128, 128], F32, kind="ExternalInput").ap()
    ones_d = nc.dram_tensor("ones128", [128, 128], F32R, kind="ExternalInput").ap()
    y_d = nc.dram_tensor("y", [TOKS_PER_CORE, D], F32, kind="ExternalOutput").ap()

    # internal DRAM for the two per-batch-element AllToAlls.
    # attn_locX rows are chunk-major: chunk j (512 rows) = my 512 head-feats
    # for 256-token group j of batch element X. After A2A, attn_gX rows are
    # global head-feats for MY 256-token slab of batch element X.
    HTB = TB // 2  # 256
    QTB = TB // 4  # 128
    attn_loc = [nc.dram_tensor("attn_loc0", [BS, HTB], F32R),
                nc.dram_tensor("attn_loc1a", [BS, QTB], F32R),
                nc.dram_tensor("attn_loc1b", [BS, QTB], F32R)]
    attn_g = [nc.dram_tensor("attn_g0", [D, HTB], F32R),
              nc.dram_tensor("attn_g1a", [D, QTB], F32R),
              nc.dram_tensor("attn_g1b", [D, QTB], F32R)]

    with tile.TileContext(nc) as tc:
        # ---- persistent tiles ---------------------------------------
        persist_cm = tc.tile_pool(name="persist", bufs=1)
        persist = persist_cm.__enter__()
        wq_sb = persist.tile([128, KC, QF], F32R, name="wq_sb")
        wk_sb = persist.tile([128, KC, HD], F32R, name="wk_sb")
        wv_sb = persist.tile([128, KC, HD], F32R, name="wv_sb")
        cos_sb = persist.tile([HD, S], F32, name="cos_sb")
        sin_sb = persist.tile([HD, S], F32, name="sin_sb")
        mask_sb = persist.tile([128, 4, TB], F32, name="mask_sb")
        rot_sb = persist.tile([HD, HD], F32R, name="rot_sb")
        ident_sb = persist.tile([128, 128], F32, name="ident_sb")
        ones_sb = persist.tile([128, 128], F32R, name="ones_sb")
        kt_sb = [persist.tile([HD, S], F32R, name=f"kt{b}_sb") for b in range(B)]
        v_sb = [persist.tile([128, JCB, HD], F32R, name=f"v{b}_sb") for b in range(B)]

        nc.sync.dma_start(wq_sb[:], wq_d.rearrange("(kc p) n -> p kc n", p=128))
        nc.sync.dma_start(wk_sb[:], wk_d.rearrange("(kc p) n -> p kc n", p=128))
        nc.sync.dma_start(wv_sb[:], wv_d.rearrange("(kc p) n -> p kc n", p=128))
        nc.sync.dma_start(cos_sb[:], cos_d[:])
        nc.sync.dma_start(sin_sb[:], sin_d[:])
        nc.sync.dma_start(mask_sb[:], mask_d[:])
        nc.sync.dma_start(rot_sb[:], rot_d[:])
        nc.sync.dma_start(ident_sb[:], ident_d[:])
        nc.sync.dma_start(ones_sb[:], ones_d[:])

        # ---- phase 1+2 pools ----------------------------------------
        p12 = []

        def pool12(*a, **kw):
            cm = tc.tile_pool(*a, **kw)
            p = cm.__enter__()
            p12.append((cm, p))
            return p

        xtp = pool12(name="xtp", bufs=3)
        qrawp = pool12(name="qrawp", bufs=2)
        vtrawp = pool12(name="vtrawp", bufs=2)
        qtp = pool12(name="qtp", bufs=6)
        tmpp = pool12(name="tmpp", bufs=3)
        ptp = pool12(name="ptp", bufs=3)
        denp = pool12(name="denp", bufs=3)
        recbp = pool12(name="recbp", bufs=1)
        atp = pool12(name="atp", bufs=2)
        asbp = pool12(name="asbp", bufs=2)
        pp = pool12(name="pp", bufs=6, space="PSUM")
        ps = pool12(name="ps", bufs=1, space="PSUM")
        pa = pool12(name="pa", bufs=1, space="PSUM")

        def emit_attention(tb, ps_pool, pa_pool, qt_tiles):
            """Generator: attention for token block tb. Yields between steps."""
            b, qb = tb // QBPB, tb % QBPB
            njc = (qb + 1) * 4
            s0 = qb * TB
            for h in range(HPC):
                denacc = denp.tile([128, TB], F32, name="denacc", tag="den")
                denf = denp.tile([128, TB], F32R, name="denf", tag="den")
                aps = pa_pool.tile([128, TB], F32, name="aps", tag="aps")
                pts = []
                for jc in range(njc):
                    sps = ps_pool.tile([128, TB], F32, name="sps", tag="sps")
                    nc.tensor.matmul(
                        sps[:], kt_sb[b][:, jc * 128:(jc + 1) * 128],
                        qt_tiles[h][:], start=True, stop=True,
                        skip_group_check=True)
                    r = jc - qb * 4
                    pt = ptp.tile([128, TB], F32R, name="pt", tag="pt")
                    if r >= 0:
                        praw = tmpp.tile([128, TB], F32, name="praw", tag="tmp")
                        nc.scalar.activation(praw[:], sps[:], AF.Exp)
                        nc.vector.tensor_tensor(pt[:], praw[:],
                                                mask_sb[:, r, :], ALU.mult)
                    else:
                        nc.scalar.activation(pt[:], sps[:], AF.Exp)
                    # denominator accumulation (fp32; final add rounds to f32r)
                    if jc == 0:
                        nc.vector.tensor_copy(denacc[:], pt[:].bitcast(F32))
                    elif jc == njc - 1:
                        nc.vector.tensor_tensor(denf[:], denacc[:],
                                                pt[:].bitcast(F32), ALU.add)
                    else:
                        nc.vector.tensor_tensor(denacc[:], denacc[:],
                                                pt[:].bitcast(F32), ALU.add)
                    pts.append(pt)
                    # AV accumulation (one step behind)
                    nc.tensor.matmul(
                        aps[:], v_sb[b][:, jc, :], pt[:],
                        start=(jc == 0), stop=(jc == njc - 1),
                        skip_group_check=True)
                    yield
                # free the attn-accumulator bank early, then
                # colsum+broadcast in one ones-matmul and a fast reciprocal
                asb = asbp.tile([128, TB], F32, name="asb", tag="asb")
                nc.vector.tensor_copy(asb[:], aps[:])
                denb = ps_pool.tile([128, TB], F32, name="denb", tag="sps")
                nc.tensor.matmul(denb[:], ones_sb[:], denf[:],
                                 start=True, stop=True, skip_group_check=True)
                recipb = recbp.tile([128, TB], F32, name="recipb", tag="recb")
                nc.vector.reciprocal_approx_fast(recipb[:], denb[:])
                attn_t = atp.tile([128, TB], F32R, name="attn_t", tag="attn_t")
                nc.vector.tensor_tensor(attn_t[:], asb[:], recipb[:], ALU.mult)
                if tb < QBPB:
                    lb = tb
                    for half in range(2):
                        nc.sync.dma_start(
                            attn_loc[0].ap()[
                                (2 * lb + half) * 512 + h * 128:
                                (2 * lb + half) * 512 + (h + 1) * 128, :],
                            attn_t[:, half * HTB:(half + 1) * HTB])
                else:
                    grp = 1 if tb < 6 else 2
                    lb = (tb - 4) % 2
                    for qt4 in range(4):
                        nc.sync.dma_start(
                            attn_loc[grp].ap()[
                                (4 * lb + qt4) * 512 + h * 128:
                                (4 * lb + qt4) * 512 + (h + 1) * 128, :],
                            attn_t[:, qt4 * QTB:(qt4 + 1) * QTB])
                yield

        def drive(gen, n):
            if gen is None:
                return None
            for _ in range(n):
                try:
                    next(gen)
                except StopIteration:
                    return None
            return gen

        prev_gen = None
        prev_steps = 0
        for tb in range(NTB):
            b, qb = tb // QBPB, tb % QBPB
            s0 = qb * TB
            per_kc = max(1, -(-prev_steps // KC))  # ceil
            # ---- projections for tb, interleaved with attention(tb-1)
            qps = [pp.tile([128, TB], F32, name=f"qps{h}", tag="proj")
                   for h in range(HPC)]
            kps = pp.tile([128, TB], F32, name="kps", tag="proj")
            vtps = pp.tile([128, TB], F32, name="vtps", tag="proj")
            for kc in range(KC):
                xt_t = xtp.tile([128, TB], F32R, name="xt_t", tag="xt")
                nc.sync.dma_start(
                    xt_t[:], xt_d[kc * 128:(kc + 1) * 128,
                                  tb * TB:(tb + 1) * TB])
                for h in range(HPC):
                    nc.tensor.matmul(
                        qps[h][:], wq_sb[:, kc, h * 128:(h + 1) * 128],
                        xt_t[:], start=(kc == 0), stop=(kc == KC - 1),
                        skip_group_check=True)
                nc.tensor.matmul(kps[:], wk_sb[:, kc, :], xt_t[:],
                                 start=(kc == 0), stop=(kc == KC - 1),
                                 skip_group_check=True)
                nc.tensor.matmul(vtps[:], wv_sb[:, kc, :], xt_t[:],
                                 start=(kc == 0), stop=(kc == KC - 1),
                                 skip_group_check=True)
                prev_gen = drive(prev_gen, per_kc)

            # ---- drains + RoPE + V transpose ------------------------
            qt_tiles = []
            for h in range(HPC):
                qraw = qrawp.tile([128, TB], F32R, name="qraw", tag="qraw")
                nc.vector.tensor_copy(qraw[:], qps[h][:])
                rotps = ps.tile([128, TB], F32, name="rotps", tag="sps")
                nc.tensor.matmul(rotps[:], rot_sb[:], qraw[:],
                                 start=True, stop=True, skip_group_check=True)
                tcos = tmpp.tile([128, TB], F32, name="tcos", tag="tmp")
                nc.vector.tensor_tensor(tcos[:], qraw[:].bitcast(F32),
                                        cos_sb[:, s0:s0 + TB], ALU.mult)
                tsin = tmpp.tile([128, TB], F32, name="tsin", tag="tmp")
                nc.vector.tensor_tensor(tsin[:], rotps[:],
                                        sin_sb[:, s0:s0 + TB], ALU.mult)
                qt = qtp.tile([128, TB], F32R, name="qt", tag="qt")
                nc.vector.tensor_tensor(qt[:], tcos[:], tsin[:], ALU.add)
                qt_tiles.append(qt)
                prev_gen = drive(prev_gen, 1)
            # K
            kraw = qrawp.tile([128, TB], F32R, name="kraw", tag="qraw")
            nc.vector.tensor_copy(kraw[:], kps[:])
            rotps = ps.tile([128, TB], F32, name="rotpsk", tag="sps")
            nc.tensor.matmul(rotps[:], rot_sb[:], kraw[:],
                             start=True, stop=True, skip_group_check=True)
            tcos = tmpp.tile([128, TB], F32, name="tcosk", tag="tmp")
            nc.vector.tensor_tensor(tcos[:], kraw[:].bitcast(F32),
                                    cos_sb[:, s0:s0 + TB], ALU.mult)
            tsin = tmpp.tile([128, TB], F32, name="tsink", tag="tmp")
            nc.vector.tensor_tensor(tsin[:], rotps[:],
                                    sin_sb[:, s0:s0 + TB], ALU.mult)
            nc.vector.tensor_tensor(kt_sb[b][:, s0:s0 + TB], tcos[:],
                                    tsin[:], ALU.add)
            # V: drain V^T then transpose 4x [128,128]
            vtraw = vtrawp.tile([128, TB], F32, name="vtraw", tag="vtraw")
            nc.vector.tensor_copy(vtraw[:], vtps[:])
            vtr = pp.tile([128, TB], F32, name="vtr", tag="proj")
            for t4 in range(4):
                nc.tensor.transpose(vtr[:, t4 * 128:(t4 + 1) * 128],
                                    vtraw[:, t4 * 128:(t4 + 1) * 128],
                                    ident_sb[:])
            nc.vector.tensor_copy(
                v_sb[b].rearrange("p jc d -> p (jc d)")[:, s0:s0 + TB],
                vtr[:])
            prev_gen = drive(prev_gen, 10 ** 9)  # flush any leftovers
            if tb in (4, 6):
                # previous collective group fully written: fire its A2A and
                # run this block's attention eagerly (SBUF-resident work)
                # so the PE has food while the collective hogs the DMA
                # engines.
                g = 0 if tb == 4 else 1
                if mock_collectives:
                    nc.sync.dma_start(attn_g[g].ap()[:], attn_loc[g].ap()[:])
                else:
                    nc.gpsimd.collective_compute(
                        "AllToAll", ALU.bypass,
                        replica_groups=[list(range(N_CORES))],
                        ins=[attn_loc[g].ap().opt()],
                        outs=[attn_g[g].ap().opt()],
                    )
                drive(emit_attention(tb, ps, pa, qt_tiles), 10 ** 9)
                prev_gen, prev_steps = None, 0
            elif tb < NTB - 1:
                prev_gen = emit_attention(tb, ps, pa, qt_tiles)
                prev_steps = HPC * ((qb + 1) * 4 + 1)
            else:
                tail_qt = qt_tiles

        # close phase-1/2 psum pools, then run the attention tail (tb=7)
        # with more generous buffering
        psum_entries = [e for e in p12 if e[1] in (pp, ps, pa)]
        for cm, p in reversed(psum_entries):
            p12.remove((cm, p))
            cm.__exit__(None, None, None)
        ps2_cm = tc.tile_pool(name="ps2", bufs=4, space="PSUM")
        ps2 = ps2_cm.__enter__()
        pa2_cm = tc.tile_pool(name="pa2", bufs=2, space="PSUM")
        pa2 = pa2_cm.__enter__()
        drive(emit_attention(NTB - 1, ps2, pa2, tail_qt), 10 ** 9)
        pa2_cm.__exit__(None, None, None)
        ps2_cm.__exit__(None, None, None)

        for cm, p in reversed(p12):
            cm.__exit__(None, None, None)
        persist_cm.__exit__(None, None, None)

        # ---- AllToAll for the last quarter --------------------------
        if mock_collectives:
            nc.sync.dma_start(attn_g[2].ap()[:], attn_loc[2].ap()[:])
        else:
            nc.gpsimd.collective_compute(
                "AllToAll", ALU.bypass,
                replica_groups=[list(range(N_CORES))],
                ins=[attn_loc[2].ap().opt()], outs=[attn_g[2].ap().opt()],
            )

        # ---- phase 3: y = attn_rows @ wo ----------------------------
        with tc.tile_pool(name="attn_sb", bufs=1) as ap3, \
             tc.tile_pool(name="wop", bufs=10) as wop, \
             tc.tile_pool(name="ysb", bufs=4) as ysbp, \
             tc.tile_pool(name="py", bufs=4, space="PSUM") as pyp:
            attn_sb = []
            for tc4 in range(4):
                t = ap3.tile([128, KC, 128], F32R, name=f"attn_sb{tc4}")
                if tc4 < 2:
                    src = attn_g[0].ap().rearrange("(hc p) q -> p hc q", p=128)[
                        :, :, tc4 * 128:(tc4 + 1) * 128]
                else:
                    src = attn_g[tc4 - 1].ap().rearrange(
                        "(hc p) q -> p hc q", p=128)
                nc.sync.dma_start(t[:], src)
                attn_sb.append(t)
            for ob in range(8):
                wo_g = []
                for g in range(8):
                    wt = wop.tile([128, 4, TB], F32R, name="wo_t", tag="wo")
                    nc.sync.dma_start(
                        wt[:], wo_d.rearrange("(hc p) n -> p hc n", p=128)
                        [:, g * 4:(g + 1) * 4, ob * TB:(ob + 1) * TB])
                    wo_g.append(wt)
                for tc4 in range(4):
                    yps = pyp.tile([128, TB], F32, name="yps", tag="yps")
                    for hc in range(KC):
                        nc.tensor.matmul(
                            yps[:], attn_sb[tc4][:, hc, :],
                            wo_g[hc // 4][:, hc % 4, :],
                            start=(hc == 0), stop=(hc == KC - 1),
                            skip_group_check=True)
                    y_sb = ysbp.tile([128, TB], F32, name="y_sb", tag="y")
                    nc.vector.tensor_copy(y_sb[:], yps[:])
                    nc.sync.dma_start(
                        y_d[tc4 * 128:(tc4 + 1) * 128,
                            ob * TB:(ob + 1) * TB], y_sb[:])

    nc.compile()
    return nc



_NC_CACHE = None


def _get_nc():
    global _NC_CACHE
    if _NC_CACHE is None:
        _NC_CACHE = build_attn_nc()
    return _NC_CACHE


def _host_reference(x, wq, wk, wv, wo, sincos, start_pos, causal_mask):
    """Numpy fallback (only used if the mask is not causal-tril)."""
    xq = (x @ wq).reshape(B, S, H, HD)
    xk = (x @ wk).reshape(B, S, KH, HD)
    xv = (x @ wv).reshape(B, S, KH, HD)
    sp = min(max(int(start_pos), 0), MS - S)
    sc = sincos[sp:sp + S]
    sin, cos = sc[:, :HD], sc[:, HD:]
    sin = sin[None, :, None, :]
    cos = cos[None, :, None, :]

    def rot(u):
        return np.concatenate([-u[..., HD // 2:], u[..., :HD // 2]], axis=-1)

    xq = xq * cos + rot(xq) * sin
    xk = xk * cos + rot(xk) * sin
    mask = np.broadcast_to(causal_mask[:, sp:sp + S, :MS], (B, S, MS))
    out = np.zeros((B, S, H, HD), dtype=np.float32)
    nrep = H // KH
    for b in range(B):
        for h in range(H):
            q = xq[b, :, h]
            k = xk[b, :, h // nrep]
            v = xv[b, :, h // nrep]
            s = (q @ k.T) * SCALE
            s = np.where(mask[b], s, -np.inf)
            s = s - s.max(axis=-1, keepdims=True)
            p = np.exp(s)
            p /= p.sum(axis=-1, keepdims=True)
            out[b, :, h] = p @ v
    return out.reshape(B, S, H * HD) @ wo


def kernel(x, wq, wk, wv, wo, cache_k, cache_v, sincos, causal_mask,
           start_pos):
    x = np.asarray(x, dtype=np.float32)
    wq = np.asarray(wq, dtype=np.float32)
    wk = np.asarray(wk, dtype=np.float32)
    wv = np.asarray(wv, dtype=np.float32)
    wo = np.asarray(wo, dtype=np.float32)
    sincos = np.asarray(sincos, dtype=np.float32)
    cm = np.asarray(causal_mask)
    sp = min(max(int(start_pos), 0), MS - S)

    tril = np.tril(np.ones((S, MS), dtype=bool))
    if not np.array_equal(cm[0, sp:sp + S, :], tril[:, :MS]):
        return _host_reference(x, wq, wk, wv, wo, sincos, start_pos,
                               cm).astype(np.float32)

    # host prep
    sc = sincos[sp:sp + S]
    sinT = np.ascontiguousarray(sc[:, :HD].T)       # [HD, S]
    cosT = np.ascontiguousarray(sc[:, HD:].T)       # [HD, S]
    xt = round_fp32r(np.ascontiguousarray(x.reshape(BS, D).T))
    wqs = wq * np.float32(SCALE)
    wo_r = round_fp32r(wo)

    maskd = np.zeros((128, 4, TB), dtype=np.float32)
    j = np.arange(128)[:, None, None]
    r = np.arange(4)[None, :, None]
    q = np.arange(TB)[None, None, :]
    maskd[(r * 128 + j) <= q] = 1.0

    rotm = np.zeros((HD, HD), dtype=np.float32)
    hh = HD // 2
    rotm[np.arange(hh) + hh, np.arange(hh)] = -1.0
    rotm[np.arange(hh), np.arange(hh) + hh] = 1.0

    ident = np.eye(128, dtype=np.float32)
    ones128 = np.ones((128, 128), dtype=np.float32)

    in_maps = []
    for c in range(N_CORES):
        in_maps.append({
            "xt": xt,
            "wq": round_fp32r(wqs[:, c * QF:(c + 1) * QF]),
            "wk": round_fp32r(wk[:, c * HD:(c + 1) * HD]),
            "wv": round_fp32r(wv[:, c * HD:(c + 1) * HD]),
            "wo": wo_r,
            "cosT": cosT, "sinT": sinT,
            "maskd": maskd, "rotm": rotm, "ident": ident,
            "ones128": ones128,
        })

    global _LAST_IN_MAPS
    _LAST_IN_MAPS = in_maps
    nc = _get_nc()
    res = run_bass_kernel_spmd(nc, in_maps, list(range(N_CORES)))
    # per-core y rows: [0:256] = b0 tokens c*256..; [256:384] = b1 tokens
    # c*128..; [384:512] = b1 tokens 1024+c*128..
    y = np.empty((BS, D), dtype=np.float32)
    for c in range(N_CORES):
        yc = res.results[c]["y"]
        y[c * 256:(c + 1) * 256] = yc[:256]
        y[S + c * 128:S + (c + 1) * 128] = yc[256:384]
        y[S + 1024 + c * 128:S + 1024 + (c + 1) * 128] = yc[384:]
    return y.reshape(B, S, D)
